# revision 1
# baseline (speedup 1.0000x reference)
"""GQA attention kernel for Trainium2, sharded over 8 NeuronCores.

Problem: B=2, S=2048, D=2048, 32 query heads / 8 KV heads, head_dim=64.
Sharding: tensor-parallel over heads — core c owns query heads [4c, 4c+4)
and KV head c (exactly one GQA group), plus the matching slices of the
projections. Each core computes a partial of the output projection
(contraction over its 256 of the 2048 Wo rows); partials are summed on host.

On-device layout is fully "transposed": x arrives as x^T [D, B*S], the
projections produce q^T/k^T (token dim on the free axis), scores are computed
as S^T = K·q^T (so the softmax denominator is a partition-dim sum, obtained
for free from a ones-column appended to V), and the output projection emits
y^T = Wo^T · out^T. Softmax skips the max-subtraction (scores/8 ~ N(0,1); exp
cannot overflow) which removes any need for partition-dim max reductions.
All matmuls run in bf16 with fp32 PSUM accumulation.
"""

import numpy as np
import ml_dtypes

import concourse.bacc as bacc
import concourse.mybir as mybir
import concourse.tile as tile
from concourse.bass_utils import run_bass_kernel_spmd
from concourse.masks import make_identity

BF16 = mybir.dt.bfloat16
F32 = mybir.dt.float32
AF = mybir.ActivationFunctionType
MULT = mybir.AluOpType.mult

B, S, D = 2, 2048, 2048
BS = B * S                    # 4096 tokens
NUM_HEADS, NUM_KV_HEADS, DH = 32, 8, 64
N_CORES = 8
HL = NUM_HEADS // N_CORES     # 4 local query heads
DQ = HL * DH                  # 256 local q dims
SC = S // 128                 # 16 key chunks per batch
NBLK = BS // 512              # 8 token blocks for projections
SCALE = 1.0 / 8.0             # 1/sqrt(64)


def build_kernel(KT, body_reps=1):
    """Build the per-core Bass program. KT = number of 128-row contraction
    tiles in the projections (16, or 17 when biases are folded in via an
    appended ones-row of x^T). body_reps emits the whole pipeline multiple
    times (benchmarking aid: Δtime between reps counts isolates HW time)."""
    nc = bacc.Bacc("TRN2", target_bir_lowering=False, debug=False,
                   num_devices=N_CORES)
    xt = nc.dram_tensor("xt", [KT * 128, BS], BF16, kind="ExternalInput").ap()
    wqkv = nc.dram_tensor("wqkv", [KT * 128, DQ + 2 * DH], BF16,
                          kind="ExternalInput").ap()
    wo = nc.dram_tensor("wo", [DQ, D], BF16, kind="ExternalInput").ap()
    yt = nc.dram_tensor("yt", [D, BS], F32, kind="ExternalOutput").ap()

    xt_r = xt.rearrange("(kt p) n -> p kt n", p=128)
    wqkv_r = wqkv.rearrange("(kt p) m -> p kt m", p=128)
    wo_r = wo.rearrange("(kt p) n -> p kt n", p=128)

    with tile.TileContext(nc) as tc:
        with tc.tile_pool(name="singles", bufs=1) as singles, \
             tc.tile_pool(name="psum", bufs=1, space="PSUM") as psum, \
             tc.tile_pool(name="sbuf", bufs=1) as sbuf:

            # --- resident SBUF tensors -----------------------------------
            wqkv_sb = singles.tile([128, KT, DQ + 2 * DH], BF16, tag="wqkv")
            nc.sync.dma_start(wqkv_sb[:], wqkv_r[:])
            wo_sb = singles.tile([128, 2, D], BF16, tag="wo")
            nc.sync.dma_start(wo_sb[:], wo_r[:])
            # q^T: m-tile 0 holds heads 0,1 (rows 0-63 / 64-127), m-tile 1
            # holds heads 2,3. k^T duplicated into both partition halves so
            # both row-groups of the packed scores matmuls can read it.
            qt_sb = singles.tile([128, 2, BS], BF16, tag="qt")
            kt2_sb = singles.tile([128, BS], BF16, tag="kt2")
            # v in natural layout [token-chunk, 65]; column 64 = 1.0 feeds
            # the softmax denominator through the AV matmul.
            v_sb = singles.tile([128, BS // 128, DH + 1], BF16, tag="v")
            nc.vector.memset(v_sb[:, :, DH:DH + 1], 1.0)
            # out^T (normalized attention output, stacked like q^T)
            ot_sb = singles.tile([128, 2, BS], BF16, tag="ot")

            ident = singles.tile([DH, DH], BF16, tag="ident")
            make_identity(nc, ident)
            ones_sb = singles.tile([1, DH], F32, tag="ones")
            nc.vector.memset(ones_sb[:], 1.0)

            # --- phase A: projections ------------------------------------
            for _rep in range(body_reps):
              with tc.tile_pool(name="xkpool", bufs=2) as xkpool:
                for blk in range(NBLK):
                    cols = slice(blk * 512, (blk + 1) * 512)
                    xk = xkpool.tile([128, KT, 512], BF16, tag="xk")
                    nc.sync.dma_start(xk[:], xt_r[:, :, cols])
                    for m in range(3):
                        pp = psum.tile([128, 512], F32, tag="mm", bufs=2)
                        msl = slice(m * 128, (m + 1) * 128)
                        for kt in range(KT):
                            nc.tensor.matmul(
                                pp[:], lhsT=wqkv_sb[:, kt, msl],
                                rhs=xk[:, kt, :],
                                start=(kt == 0), stop=(kt == KT - 1))
                        if m < 2:
                            nc.scalar.copy(out=qt_sb[:, m, cols], in_=pp[:])
                        else:
                            # rows 0-63: k^T (duplicate to both halves);
                            # rows 64-127: v^T (transpose into v_sb)
                            nc.scalar.copy(out=kt2_sb[0:64, cols],
                                           in_=pp[0:64, :])
                            nc.scalar.copy(out=kt2_sb[64:128, cols],
                                           in_=pp[0:64, :])
                            vt = sbuf.tile([64, 512], BF16, tag="vt", bufs=2)
                            nc.scalar.copy(out=vt[:], in_=pp[64:128, :])
                            for j in range(4):
                                pt = psum.tile([128, DH], BF16, tag="misc")
                                nc.tensor.transpose(
                                    pt[:], vt[:, j * 128:(j + 1) * 128],
                                    ident[:])
                                nc.vector.tensor_copy(
                                    out=v_sb[:, blk * 4 + j, 0:DH],
                                    in_=pt[:])

              # --- phases B & C: attention + output projection -----------
              with tc.tile_pool(name="exppool", bufs=2) as exppool:
                for pair in range(4):
                    b, m = pair // 2, pair % 2
                    base = b * S
                    for qb in range(4):
                        qcols = slice(base + qb * 512, base + (qb + 1) * 512)
                        exps = [exppool.tile([128, SC, 512], BF16,
                                             tag=f"exp{h}", name=f"exp{h}")
                                for h in range(2)]
                        avs = [psum.tile([128, 512], F32, tag="av", bufs=2,
                                         name=f"av{h}") for h in range(2)]
                        for kt in range(SC):
                            kr = slice(base + kt * 128, base + (kt + 1) * 128)
                            for h in range(2):
                                rows = slice(h * 64, (h + 1) * 64)
                                sc = psum.tile([128, 512], F32, tag="sc",
                                               bufs=3)
                                nc.tensor.matmul(
                                    sc[:], lhsT=kt2_sb[rows, kr],
                                    rhs=qt_sb[rows, m, qcols],
                                    start=True, stop=True)
                                nc.scalar.activation(
                                    out=exps[h][:, kt, :], in_=sc[:],
                                    func=AF.Exp, scale=SCALE)
                                nc.tensor.matmul(
                                    avs[h][0:DH + 1],
                                    lhsT=v_sb[:, b * SC + kt, :],
                                    rhs=exps[h][:, kt, :],
                                    start=(kt == 0), stop=(kt == SC - 1))
                        # normalization: denominators sit in row 64 of each
                        # AV accumulator; broadcast 1/denom across the 64
                        # partitions via a K=1 ones outer-product on PE.
                        bcp = psum.tile([128, 512], F32, tag="misc")
                        for h in range(2):
                            den = sbuf.tile([1, 512], F32, tag="den", bufs=2)
                            nc.vector.tensor_copy(out=den[:],
                                                  in_=avs[h][DH:DH + 1, :])
                            rec = sbuf.tile([1, 512], F32, tag="rec", bufs=2)
                            nc.vector.reciprocal_approx_fast(rec[:], den[:])
                            nc.tensor.matmul(
                                bcp[h * 64:(h + 1) * 64, :],
                                lhsT=ones_sb[:], rhs=rec[:],
                                start=True, stop=True)
                        bcs = sbuf.tile([128, 512], F32, tag="bcs", bufs=2)
                        nc.vector.tensor_copy(out=bcs[:], in_=bcp[:])
                        for h in range(2):
                            rows = slice(h * 64, (h + 1) * 64)
                            nc.vector.tensor_tensor(
                                out=ot_sb[rows, m, qcols],
                                in0=avs[h][0:64, :], in1=bcs[rows, :],
                                op=MULT)
                    if m == 1:
                        # output projection for batch b (all 4 heads ready)
                        for dm in range(16):
                            dsl = slice(dm * 128, (dm + 1) * 128)
                            for qb in range(4):
                                qcols = slice(base + qb * 512,
                                              base + (qb + 1) * 512)
                                po = psum.tile([128, 512], F32, tag="mm",
                                               bufs=2)
                                for kt in range(2):
                                    nc.tensor.matmul(
                                        po[:], lhsT=wo_sb[:, kt, dsl],
                                        rhs=ot_sb[:, kt, qcols],
                                        start=(kt == 0), stop=(kt == 1))
                                oc = sbuf.tile([128, 512], F32, tag="oc",
                                               bufs=4)
                                nc.vector.tensor_copy(out=oc[:], in_=po[:])
                                nc.sync.dma_start(yt[dsl, qcols], oc[:])
    nc.compile()
    return nc


_CACHE = {}


def _get_kernel(KT):
    if KT not in _CACHE:
        _CACHE[KT] = build_kernel(KT)
    return _CACHE[KT]


def kernel(x, Wq, bq, Wk, bk, Wv, bv, Wo, bo):
    x = np.asarray(x, dtype=np.float32)
    Wq = np.asarray(Wq, dtype=np.float32)
    Wk = np.asarray(Wk, dtype=np.float32)
    Wv = np.asarray(Wv, dtype=np.float32)
    Wo = np.asarray(Wo, dtype=np.float32)
    bq = np.asarray(bq, dtype=np.float32)
    bk = np.asarray(bk, dtype=np.float32)
    bv = np.asarray(bv, dtype=np.float32)
    bo = np.asarray(bo, dtype=np.float32)
    assert x.shape == (B, S, D)

    has_bias = bool(np.any(bq) or np.any(bk) or np.any(bv))
    KT = 17 if has_bias else 16

    # x^T [D, B*S] (+ ones row when biases are folded into the projections)
    xt = np.zeros((KT * 128, BS), dtype=ml_dtypes.bfloat16)
    xt[:D] = x.transpose(2, 0, 1).reshape(D, BS).astype(ml_dtypes.bfloat16)
    if has_bias:
        xt[D] = 1.0

    nc = _get_kernel(KT)
    in_maps = []
    for c in range(N_CORES):
        qsl = slice(c * DQ, (c + 1) * DQ)
        ksl = slice(c * DH, (c + 1) * DH)
        wqkv = np.zeros((KT * 128, DQ + 2 * DH), dtype=ml_dtypes.bfloat16)
        wqkv[:D, :DQ] = Wq[:, qsl].astype(ml_dtypes.bfloat16)
        wqkv[:D, DQ:DQ + DH] = Wk[:, ksl].astype(ml_dtypes.bfloat16)
        wqkv[:D, DQ + DH:] = Wv[:, ksl].astype(ml_dtypes.bfloat16)
        if has_bias:
            wqkv[D, :DQ] = bq[qsl].astype(ml_dtypes.bfloat16)
            wqkv[D, DQ:DQ + DH] = bk[ksl].astype(ml_dtypes.bfloat16)
            wqkv[D, DQ + DH:] = bv[ksl].astype(ml_dtypes.bfloat16)
        in_maps.append({
            "xt": xt,
            "wqkv": wqkv,
            "wo": np.ascontiguousarray(Wo[qsl]).astype(ml_dtypes.bfloat16),
        })

    res = run_bass_kernel_spmd(nc, in_maps, core_ids=list(range(N_CORES)))
    yt = np.zeros((D, BS), dtype=np.float32)
    for r in res.results:
        yt += r["yt"]
    y = yt.reshape(D, B, S).transpose(1, 2, 0) + bo
    return np.ascontiguousarray(y, dtype=np.float32)



# revision 4
# speedup vs baseline: 1.1832x; 1.1832x over previous
"""GQA attention kernel for Trainium2, sharded over 8 NeuronCores.

Problem: B=2, S=2048, D=2048, 32 query heads / 8 KV heads, head_dim=64.
Sharding: tensor-parallel over heads — core c owns query heads [4c, 4c+4)
and KV head c (exactly one GQA group), plus the matching slices of the
projections. Each core computes a partial of the output projection
(contraction over its 256 of the 2048 Wo rows); partials are summed on host.

On-device layout is fully "transposed": x arrives as x^T [D, B*S], the
projections produce q^T/k^T (token dim on the free axis), scores are computed
as S^T = K·q^T (so the softmax denominator is a partition-dim sum, obtained
for free from a ones-column appended to V), and the output projection emits
y^T = Wo^T · out^T. Softmax skips the max-subtraction (scores/8 ~ N(0,1); exp
cannot overflow). All matmuls run in bf16 with fp32 PSUM accumulation.

Schedule (v2): software-pipelined across batches — projections for batch 1
are emitted inside batch 0's attention, and each batch's output projection is
emitted inside its own m=1 attention pass. The two scores matmuls of a head
pair are adjacent 64x128 row-tiles (T0/T8) so they run concurrently on the PE
array; exp is one [128,1024] ACT instruction per key chunk covering both
heads. PSUM budget: sc 2x2 banks + av 2 + mm 2 = 8.
"""

import numpy as np
import ml_dtypes

import concourse.bacc as bacc
import concourse.mybir as mybir
import concourse.tile as tile
from concourse.bass_utils import run_bass_kernel_spmd
from concourse.masks import make_identity

BF16 = mybir.dt.bfloat16
F32 = mybir.dt.float32
AF = mybir.ActivationFunctionType
MULT = mybir.AluOpType.mult

B, S, D = 2, 2048, 2048
BS = B * S                    # 4096 tokens
NUM_HEADS, NUM_KV_HEADS, DH = 32, 8, 64
N_CORES = 8
HL = NUM_HEADS // N_CORES     # 4 local query heads
DQ = HL * DH                  # 256 local q dims
SC = S // 128                 # 16 key chunks per batch
SCALE = 1.0 / 8.0             # 1/sqrt(64)


def build_kernel(KT):
    """Build the per-core Bass program. KT = number of 128-row contraction
    tiles in the projections (16, or 17 when biases are folded in via an
    appended ones-row of x^T)."""
    nc = bacc.Bacc("TRN2", target_bir_lowering=False, debug=False,
                   num_devices=N_CORES)
    xt = nc.dram_tensor("xt", [KT * 128, BS], BF16, kind="ExternalInput").ap()
    wqkv = nc.dram_tensor("wqkv", [KT * 128, DQ + 2 * DH], BF16,
                          kind="ExternalInput").ap()
    wo = nc.dram_tensor("wo", [DQ, D], BF16, kind="ExternalInput").ap()
    yt = nc.dram_tensor("yt", [D, BS], BF16, kind="ExternalOutput").ap()

    xt_r = xt.rearrange("(kt p) n -> p kt n", p=128)
    wqkv_r = wqkv.rearrange("(kt p) m -> p kt m", p=128)
    wo_r = wo.rearrange("(kt p) n -> p kt n", p=128)

    with tile.TileContext(nc) as tc:
        with tc.tile_pool(name="singles", bufs=1) as singles, \
             tc.tile_pool(name="psum", bufs=1, space="PSUM") as psum, \
             tc.tile_pool(name="sbuf", bufs=1) as sbuf, \
             tc.tile_pool(name="xkpool", bufs=2) as xkpool, \
             tc.tile_pool(name="exppool", bufs=2) as exppool:

            # --- resident SBUF tensors -----------------------------------
            wqkv_sb = singles.tile([128, KT, DQ + 2 * DH], BF16, tag="wqkv")
            nc.sync.dma_start(wqkv_sb[:], wqkv_r[:])
            wo_sb = singles.tile([128, 2, D], BF16, tag="wo")
            nc.sync.dma_start(wo_sb[:], wo_r[:])

            # per-batch tensors (statically double-allocated)
            # q^T: m-tile 0 holds heads 0,1 (rows 0-63 / 64-127), m-tile 1
            # holds heads 2,3. k^T duplicated into both partition halves so
            # both row-tiles of the paired scores matmuls can read it.
            qt = [singles.tile([128, 2, S], BF16, tag=f"qt{b}",
                               name=f"qt{b}") for b in range(B)]
            kt2 = [singles.tile([128, S], BF16, tag=f"kt2{b}",
                                name=f"kt2{b}") for b in range(B)]
            # v in natural layout [token-chunk, 65]; column 64 = 1.0 feeds
            # the softmax denominator through the AV matmul.
            v_sb = [singles.tile([128, SC, DH + 1], BF16, tag=f"v{b}",
                                 name=f"v{b}") for b in range(B)]
            ot = [singles.tile([128, 2, S], BF16, tag=f"ot{b}",
                               name=f"ot{b}") for b in range(B)]
            for b in range(B):
                nc.vector.memset(v_sb[b][:, :, DH:DH + 1], 1.0)

            ident = singles.tile([DH, DH], BF16, tag="ident")
            make_identity(nc, ident)
            ones_sb = singles.tile([1, DH], F32, tag="ones")
            nc.vector.memset(ones_sb[:], 1.0)

            # --- emission helpers ----------------------------------------
            def emit_proj_block(b, blk):
                """Projections for one 512-token block of batch b."""
                gcols = slice(b * S + blk * 512, b * S + (blk + 1) * 512)
                cols = slice(blk * 512, (blk + 1) * 512)
                xk = xkpool.tile([128, KT, 512], BF16, tag="xk")
                nc.sync.dma_start(xk[:], xt_r[:, :, gcols])
                for m in range(3):
                    pp = psum.tile([128, 512], F32, tag="mm", bufs=2)
                    msl = slice(m * 128, (m + 1) * 128)
                    for kt in range(KT):
                        nc.tensor.matmul(
                            pp[:], lhsT=wqkv_sb[:, kt, msl],
                            rhs=xk[:, kt, :],
                            start=(kt == 0), stop=(kt == KT - 1))
                    if m < 2:
                        nc.vector.tensor_copy(out=qt[b][:, m, cols],
                                              in_=pp[:])
                    else:
                        # rows 0-63: k^T (duplicate to both halves);
                        # rows 64-127: v^T (transpose into v_sb)
                        nc.vector.tensor_copy(out=kt2[b][0:64, cols],
                                              in_=pp[0:64, :])
                        nc.vector.tensor_copy(out=kt2[b][64:128, cols],
                                              in_=pp[0:64, :])
                        vt = sbuf.tile([64, 512], BF16, tag="vt", bufs=2)
                        nc.vector.tensor_copy(out=vt[:], in_=pp[64:128, :])
                        for j in range(4):
                            pt = psum.tile([128, DH], BF16, tag="mm",
                                           bufs=2)
                            nc.tensor.transpose(
                                pt[:], vt[:, j * 128:(j + 1) * 128],
                                ident[:])
                            nc.vector.tensor_copy(
                                out=v_sb[b][:, blk * 4 + j, 0:DH],
                                in_=pt[:])

            def emit_attn_unit(b, m, qb):
                """Attention for 512 queries of head-pair m, batch b."""
                qcols = slice(qb * 512, (qb + 1) * 512)
                exps = exppool.tile([128, SC, 2, 512], BF16, tag="exps")
                avs = [psum.tile([128, 512], F32, tag="av", bufs=2,
                                 name=f"av{h}") for h in range(2)]
                def emit_av(kt):
                    for h in range(2):
                        nc.tensor.matmul(
                            avs[h][0:DH + 1],
                            lhsT=v_sb[b][:, kt, :],
                            rhs=exps[:, kt, h, :],
                            start=(kt == 0), stop=(kt == SC - 1))

                for kt in range(SC):
                    kr = slice(kt * 128, (kt + 1) * 128)
                    # paired scores matmuls: h0 on rows 0-63 (tile T0),
                    # h1 on rows 64-127 (tile T8) — concurrent row-tiles.
                    sc = psum.tile([128, 2, 512], F32, tag="sc", bufs=2)
                    for h in range(2):
                        rows = slice(h * 64, (h + 1) * 64)
                        nc.tensor.matmul(
                            sc[:, h, :], lhsT=kt2[b][rows, kr],
                            rhs=qt[b][rows, m, qcols],
                            start=True, stop=True)
                    # one exp for both heads: [128, 1024] PSUM -> SBUF
                    nc.scalar.activation(
                        out=exps[:, kt, :, :], in_=sc[:, :, :],
                        func=AF.Exp, scale=SCALE)
                    # AV lags one chunk so the PE FIFO never blocks on the
                    # exp it just requested.
                    if kt > 0:
                        emit_av(kt - 1)
                emit_av(SC - 1)
                # normalization: denominators sit in row 64 of each AV
                # accumulator; broadcast 1/denom across the 64 partitions
                # via a K=1 ones outer-product on PE.
                bcp = psum.tile([128, 512], F32, tag="mm", bufs=2)
                for h in range(2):
                    den = sbuf.tile([1, 512], F32, tag="den", bufs=2)
                    nc.vector.tensor_copy(out=den[:],
                                          in_=avs[h][DH:DH + 1, :])
                    rec = sbuf.tile([1, 512], F32, tag="rec", bufs=2)
                    nc.vector.reciprocal_approx_fast(rec[:], den[:])
                    nc.tensor.matmul(
                        bcp[h * 64:(h + 1) * 64, :],
                        lhsT=ones_sb[:], rhs=rec[:],
                        start=True, stop=True)
                bcs = sbuf.tile([128, 512], F32, tag="bcs", bufs=2)
                nc.vector.tensor_copy(out=bcs[:], in_=bcp[:])
                for h in range(2):
                    rows = slice(h * 64, (h + 1) * 64)
                    nc.vector.tensor_tensor(
                        out=ot[b][rows, m, qcols],
                        in0=avs[h][0:64, :], in1=bcs[rows, :],
                        op=MULT)

            def emit_oproj(b, qb):
                """Output projection for 512 queries of batch b (needs both
                head-pairs of this query block in ot)."""
                qcols = slice(qb * 512, (qb + 1) * 512)
                gcols = slice(b * S + qb * 512, b * S + (qb + 1) * 512)
                for dm in range(16):
                    dsl = slice(dm * 128, (dm + 1) * 128)
                    po = psum.tile([128, 512], F32, tag="mm", bufs=2)
                    for kt in range(2):
                        nc.tensor.matmul(
                            po[:], lhsT=wo_sb[:, kt, dsl],
                            rhs=ot[b][:, kt, qcols],
                            start=(kt == 0), stop=(kt == 1))
                    oc = sbuf.tile([128, 512], BF16, tag="oc", bufs=4)
                    nc.vector.tensor_copy(out=oc[:], in_=po[:])
                    nc.sync.dma_start(yt[dsl, gcols], oc[:])

            # --- pipelined schedule --------------------------------------
            for blk in range(4):
                emit_proj_block(0, blk)
            for qb in range(4):
                emit_attn_unit(0, 0, qb)
                emit_proj_block(1, qb)
            for qb in range(4):
                emit_attn_unit(0, 1, qb)
                emit_oproj(0, qb)
            for qb in range(4):
                emit_attn_unit(1, 0, qb)
            for qb in range(4):
                emit_attn_unit(1, 1, qb)
                emit_oproj(1, qb)
    nc.compile()
    return nc


_CACHE = {}


def _get_kernel(KT):
    if KT not in _CACHE:
        _CACHE[KT] = build_kernel(KT)
    return _CACHE[KT]


def kernel(x, Wq, bq, Wk, bk, Wv, bv, Wo, bo):
    x = np.asarray(x, dtype=np.float32)
    Wq = np.asarray(Wq, dtype=np.float32)
    Wk = np.asarray(Wk, dtype=np.float32)
    Wv = np.asarray(Wv, dtype=np.float32)
    Wo = np.asarray(Wo, dtype=np.float32)
    bq = np.asarray(bq, dtype=np.float32)
    bk = np.asarray(bk, dtype=np.float32)
    bv = np.asarray(bv, dtype=np.float32)
    bo = np.asarray(bo, dtype=np.float32)
    assert x.shape == (B, S, D)

    has_bias = bool(np.any(bq) or np.any(bk) or np.any(bv))
    KT = 17 if has_bias else 16

    # x^T [D, B*S] (+ ones row when biases are folded into the projections)
    xt = np.zeros((KT * 128, BS), dtype=ml_dtypes.bfloat16)
    xt[:D] = x.transpose(2, 0, 1).reshape(D, BS).astype(ml_dtypes.bfloat16)
    if has_bias:
        xt[D] = 1.0

    nc = _get_kernel(KT)
    in_maps = []
    for c in range(N_CORES):
        qsl = slice(c * DQ, (c + 1) * DQ)
        ksl = slice(c * DH, (c + 1) * DH)
        wqkv = np.zeros((KT * 128, DQ + 2 * DH), dtype=ml_dtypes.bfloat16)
        wqkv[:D, :DQ] = Wq[:, qsl].astype(ml_dtypes.bfloat16)
        wqkv[:D, DQ:DQ + DH] = Wk[:, ksl].astype(ml_dtypes.bfloat16)
        wqkv[:D, DQ + DH:] = Wv[:, ksl].astype(ml_dtypes.bfloat16)
        if has_bias:
            wqkv[D, :DQ] = bq[qsl].astype(ml_dtypes.bfloat16)
            wqkv[D, DQ:DQ + DH] = bk[ksl].astype(ml_dtypes.bfloat16)
            wqkv[D, DQ + DH:] = bv[ksl].astype(ml_dtypes.bfloat16)
        in_maps.append({
            "xt": xt,
            "wqkv": wqkv,
            "wo": np.ascontiguousarray(Wo[qsl]).astype(ml_dtypes.bfloat16),
        })

    res = run_bass_kernel_spmd(nc, in_maps, core_ids=list(range(N_CORES)))
    yt = np.zeros((D, BS), dtype=np.float32)
    for r in res.results:
        yt += np.asarray(r["yt"], dtype=np.float32)
    y = yt.reshape(D, B, S).transpose(1, 2, 0) + bo
    return np.ascontiguousarray(y, dtype=np.float32)


# revision 8
# speedup vs baseline: 1.3450x; 1.1367x over previous
"""GQA attention kernel for Trainium2, sharded over 8 NeuronCores.

Problem: B=2, S=2048, D=2048, 32 query heads / 8 KV heads, head_dim=64.
Sharding: tensor-parallel over heads — core c owns query heads [4c, 4c+4)
and KV head c (exactly one GQA group), plus the matching slices of the
projections. Each core computes a partial of the output projection
(contraction over its 256 of the 2048 Wo rows); partials are summed on host.

On-device layout is fully "transposed": x arrives as x^T [D, B*S], the
projections produce q^T/k^T (token dim on the free axis), scores are computed
as S^T = K·q^T (so the softmax denominator is a partition-dim sum, obtained
for free from a ones-column appended to V), and the output projection emits
y^T = Wo^T · out^T. Softmax skips the max-subtraction (scores/8 ~ N(0,1); exp
cannot overflow). All matmuls run in bf16 with fp32 PSUM accumulation.

Schedule (v2): software-pipelined across batches — projections for batch 1
are emitted inside batch 0's attention, and each batch's output projection is
emitted inside its own m=1 attention pass. The two scores matmuls of a head
pair are adjacent 64x128 row-tiles (T0/T8) so they run concurrently on the PE
array; exp is one [128,1024] ACT instruction per key chunk covering both
heads. PSUM budget: sc 2x2 banks + av 2 + mm 2 = 8.
"""

import numpy as np
import ml_dtypes

import concourse.bacc as bacc
import concourse.mybir as mybir
import concourse.tile as tile
from concourse.bass_utils import run_bass_kernel_spmd
from concourse.masks import make_identity

BF16 = mybir.dt.bfloat16
F32 = mybir.dt.float32
AF = mybir.ActivationFunctionType
MULT = mybir.AluOpType.mult

B, S, D = 2, 2048, 2048
BS = B * S                    # 4096 tokens
NUM_HEADS, NUM_KV_HEADS, DH = 32, 8, 64
N_CORES = 8
HL = NUM_HEADS // N_CORES     # 4 local query heads
DQ = HL * DH                  # 256 local q dims
SC = S // 128                 # 16 key chunks per batch
SCALE = 1.0 / 8.0             # 1/sqrt(64)


def build_kernel(KT):
    """Build the per-core Bass program. KT = number of 128-row contraction
    tiles in the projections (16, or 17 when biases are folded in via an
    appended ones-row of x^T)."""
    nc = bacc.Bacc("TRN2", target_bir_lowering=False, debug=False,
                   num_devices=N_CORES)
    xt = nc.dram_tensor("xt", [KT * 128, BS], BF16, kind="ExternalInput").ap()
    wqkv = nc.dram_tensor("wqkv", [KT * 128, DQ + 2 * DH], BF16,
                          kind="ExternalInput").ap()
    wo = nc.dram_tensor("wo", [DQ, D], BF16, kind="ExternalInput").ap()
    yt = nc.dram_tensor("yt", [D, BS], BF16, kind="ExternalOutput").ap()

    xt_r = xt.rearrange("(kt p) n -> p kt n", p=128)
    wqkv_r = wqkv.rearrange("(kt p) m -> p kt m", p=128)
    wo_r = wo.rearrange("(kt p) n -> p kt n", p=128)

    with tile.TileContext(nc) as tc:
        with tc.tile_pool(name="singles", bufs=1) as singles, \
             tc.tile_pool(name="psum", bufs=1, space="PSUM") as psum, \
             tc.tile_pool(name="sbuf", bufs=1) as sbuf, \
             tc.tile_pool(name="xkpool", bufs=2) as xkpool, \
             tc.tile_pool(name="exppool", bufs=2) as exppool:

            # --- resident SBUF tensors -----------------------------------
            wqkv_sb = singles.tile([128, KT, DQ + 2 * DH], BF16, tag="wqkv")
            nc.sync.dma_start(wqkv_sb[:], wqkv_r[:])
            wo_sb = singles.tile([128, 2, D], BF16, tag="wo")
            nc.sync.dma_start(wo_sb[:], wo_r[:])

            # per-batch tensors (statically double-allocated)
            # q^T: m-tile 0 holds heads 0,1 (rows 0-63 / 64-127), m-tile 1
            # holds heads 2,3. k^T duplicated into both partition halves so
            # both row-tiles of the paired scores matmuls can read it.
            qt = [singles.tile([128, 2, S], BF16, tag=f"qt{b}",
                               name=f"qt{b}") for b in range(B)]
            kt2 = [singles.tile([128, S], BF16, tag=f"kt2{b}",
                                name=f"kt2{b}") for b in range(B)]
            # v in natural layout [token-chunk, 128]; columns 64-127 are all
            # 1.0, so the AV matmul lands 64 broadcast copies of the softmax
            # denominator on PSUM partitions 64-127 — normalization then
            # needs no PE broadcast, just a DVE reciprocal + multiply.
            v_sb = [singles.tile([128, SC, 128], BF16, tag=f"v{b}",
                                 name=f"v{b}") for b in range(B)]
            ot = [singles.tile([128, 2, S], BF16, tag=f"ot{b}",
                               name=f"ot{b}") for b in range(B)]
            for b in range(B):
                nc.vector.memset(v_sb[b][:, :, DH:], 1.0)

            ident = singles.tile([DH, DH], BF16, tag="ident")
            make_identity(nc, ident)

            # --- emission helpers ----------------------------------------
            def emit_proj_block(b, blk):
                """Projections for one 512-token block of batch b."""
                gcols = slice(b * S + blk * 512, b * S + (blk + 1) * 512)
                cols = slice(blk * 512, (blk + 1) * 512)
                xk = xkpool.tile([128, KT, 512], BF16, tag="xk")
                nc.sync.dma_start(xk[:], xt_r[:, :, gcols])
                for m in range(3):
                    pp = psum.tile([128, 512], F32, tag="mm", bufs=2)
                    msl = slice(m * 128, (m + 1) * 128)
                    for kt in range(KT):
                        nc.tensor.matmul(
                            pp[:], lhsT=wqkv_sb[:, kt, msl],
                            rhs=xk[:, kt, :],
                            start=(kt == 0), stop=(kt == KT - 1))
                    if m < 2:
                        nc.vector.tensor_copy(out=qt[b][:, m, cols],
                                              in_=pp[:])
                    else:
                        # rows 0-63: k^T (duplicate to both halves);
                        # rows 64-127: v^T (transpose into v_sb)
                        nc.vector.tensor_copy(out=kt2[b][0:64, cols],
                                              in_=pp[0:64, :])
                        nc.vector.tensor_copy(out=kt2[b][64:128, cols],
                                              in_=pp[0:64, :])
                        vt = sbuf.tile([64, 512], BF16, tag="vt", bufs=2)
                        nc.vector.tensor_copy(out=vt[:], in_=pp[64:128, :])
                        for j in range(4):
                            pt = psum.tile([128, DH], BF16, tag="mm",
                                           bufs=2)
                            nc.tensor.transpose(
                                pt[:], vt[:, j * 128:(j + 1) * 128],
                                ident[:])
                            nc.vector.tensor_copy(
                                out=v_sb[b][:, blk * 4 + j, 0:DH],
                                in_=pt[:])

            def emit_attn_unit(b, m, qb):
                """Attention for 512 queries of head-pair m, batch b."""
                qcols = slice(qb * 512, (qb + 1) * 512)
                exps = exppool.tile([128, SC, 2, 512], BF16, tag="exps")
                avs = [psum.tile([128, 512], F32, tag="av", bufs=2,
                                 name=f"av{h}") for h in range(2)]
                def emit_av(kt):
                    for h in range(2):
                        nc.tensor.matmul(
                            avs[h][:],
                            lhsT=v_sb[b][:, kt, :],
                            rhs=exps[:, kt, h, :],
                            start=(kt == 0), stop=(kt == SC - 1))

                for kt in range(SC):
                    kr = slice(kt * 128, (kt + 1) * 128)
                    # paired scores matmuls: h0 on rows 0-63 (tile T0),
                    # h1 on rows 64-127 (tile T8) — concurrent row-tiles.
                    sc = psum.tile([128, 2, 512], F32, tag="sc", bufs=2)
                    for h in range(2):
                        rows = slice(h * 64, (h + 1) * 64)
                        nc.tensor.matmul(
                            sc[:, h, :], lhsT=kt2[b][rows, kr],
                            rhs=qt[b][rows, m, qcols],
                            start=True, stop=True)
                    # one exp for both heads: [128, 1024] PSUM -> SBUF
                    nc.scalar.activation(
                        out=exps[:, kt, :, :], in_=sc[:, :, :],
                        func=AF.Exp, scale=SCALE)
                    # AV lags one chunk so the PE FIFO never blocks on the
                    # exp it just requested.
                    if kt > 0:
                        emit_av(kt - 1)
                emit_av(SC - 1)
                # normalization: 64 broadcast copies of the denominator sit
                # on partitions 64-127 of each AV accumulator.
                for h in range(2):
                    rows = slice(h * 64, (h + 1) * 64)
                    rec = sbuf.tile([64, 512], F32, tag="rec", bufs=2,
                                    name=f"rec{h}")
                    nc.vector.reciprocal_approx_fast(rec[:],
                                                     avs[h][64:128, :])
                    nc.vector.tensor_tensor(
                        out=ot[b][rows, m, qcols],
                        in0=avs[h][0:64, :], in1=rec[:],
                        op=MULT)

            def emit_oproj(b, qb):
                """Output projection for 512 queries of batch b (needs both
                head-pairs of this query block in ot)."""
                qcols = slice(qb * 512, (qb + 1) * 512)
                gcols = slice(b * S + qb * 512, b * S + (qb + 1) * 512)
                for dm in range(16):
                    dsl = slice(dm * 128, (dm + 1) * 128)
                    po = psum.tile([128, 512], F32, tag="mm", bufs=2)
                    for kt in range(2):
                        nc.tensor.matmul(
                            po[:], lhsT=wo_sb[:, kt, dsl],
                            rhs=ot[b][:, kt, qcols],
                            start=(kt == 0), stop=(kt == 1))
                    oc = sbuf.tile([128, 512], BF16, tag="oc", bufs=4)
                    nc.vector.tensor_copy(out=oc[:], in_=po[:])
                    nc.sync.dma_start(yt[dsl, gcols], oc[:])

            # --- pipelined schedule --------------------------------------
            for blk in range(4):
                emit_proj_block(0, blk)
            for qb in range(4):
                emit_attn_unit(0, 0, qb)
                emit_proj_block(1, qb)
            for qb in range(4):
                emit_attn_unit(0, 1, qb)
                emit_oproj(0, qb)
            for qb in range(4):
                emit_attn_unit(1, 0, qb)
            for qb in range(4):
                emit_attn_unit(1, 1, qb)
                emit_oproj(1, qb)
    nc.compile()
    return nc


_CACHE = {}


def _get_kernel(KT):
    if KT not in _CACHE:
        _CACHE[KT] = build_kernel(KT)
    return _CACHE[KT]


def kernel(x, Wq, bq, Wk, bk, Wv, bv, Wo, bo):
    x = np.asarray(x, dtype=np.float32)
    Wq = np.asarray(Wq, dtype=np.float32)
    Wk = np.asarray(Wk, dtype=np.float32)
    Wv = np.asarray(Wv, dtype=np.float32)
    Wo = np.asarray(Wo, dtype=np.float32)
    bq = np.asarray(bq, dtype=np.float32)
    bk = np.asarray(bk, dtype=np.float32)
    bv = np.asarray(bv, dtype=np.float32)
    bo = np.asarray(bo, dtype=np.float32)
    assert x.shape == (B, S, D)

    has_bias = bool(np.any(bq) or np.any(bk) or np.any(bv))
    KT = 17 if has_bias else 16

    # x^T [D, B*S] (+ ones row when biases are folded into the projections)
    xt = np.zeros((KT * 128, BS), dtype=ml_dtypes.bfloat16)
    xt[:D] = x.transpose(2, 0, 1).reshape(D, BS).astype(ml_dtypes.bfloat16)
    if has_bias:
        xt[D] = 1.0

    nc = _get_kernel(KT)
    in_maps = []
    for c in range(N_CORES):
        qsl = slice(c * DQ, (c + 1) * DQ)
        ksl = slice(c * DH, (c + 1) * DH)
        wqkv = np.zeros((KT * 128, DQ + 2 * DH), dtype=ml_dtypes.bfloat16)
        wqkv[:D, :DQ] = Wq[:, qsl].astype(ml_dtypes.bfloat16)
        wqkv[:D, DQ:DQ + DH] = Wk[:, ksl].astype(ml_dtypes.bfloat16)
        wqkv[:D, DQ + DH:] = Wv[:, ksl].astype(ml_dtypes.bfloat16)
        if has_bias:
            wqkv[D, :DQ] = bq[qsl].astype(ml_dtypes.bfloat16)
            wqkv[D, DQ:DQ + DH] = bk[ksl].astype(ml_dtypes.bfloat16)
            wqkv[D, DQ + DH:] = bv[ksl].astype(ml_dtypes.bfloat16)
        in_maps.append({
            "xt": xt,
            "wqkv": wqkv,
            "wo": np.ascontiguousarray(Wo[qsl]).astype(ml_dtypes.bfloat16),
        })

    res = run_bass_kernel_spmd(nc, in_maps, core_ids=list(range(N_CORES)))
    yt = np.zeros((D, BS), dtype=np.float32)
    for r in res.results:
        yt += np.asarray(r["yt"], dtype=np.float32)
    y = yt.reshape(D, B, S).transpose(1, 2, 0) + bo
    return np.ascontiguousarray(y, dtype=np.float32)


# revision 14
# speedup vs baseline: 1.3475x; 1.0018x over previous
"""GQA attention kernel for Trainium2, sharded over 8 NeuronCores.

Problem: B=2, S=2048, D=2048, 32 query heads / 8 KV heads, head_dim=64.
Sharding: tensor-parallel over heads — core c owns query heads [4c, 4c+4)
and KV head c (exactly one GQA group), plus the matching slices of the
projections. Each core computes a partial of the output projection
(contraction over its 256 of the 2048 Wo rows); partials are summed on host.

On-device layout is fully "transposed": x arrives as x^T [D, B*S], the
projections produce q^T/k^T (token dim on the free axis), scores are computed
as S^T = K·q^T (so the softmax denominator is a partition-dim sum, obtained
for free from a ones-column appended to V), and the output projection emits
y^T = Wo^T · out^T. Softmax skips the max-subtraction (scores/8 ~ N(0,1); exp
cannot overflow). All matmuls run in bf16 with fp32 PSUM accumulation.

Schedule (v2): software-pipelined across batches — projections for batch 1
are emitted inside batch 0's attention, and each batch's output projection is
emitted inside its own m=1 attention pass. The two scores matmuls of a head
pair are adjacent 64x128 row-tiles (T0/T8) so they run concurrently on the PE
array; exp is one [128,1024] ACT instruction per key chunk covering both
heads. PSUM budget: sc 2x2 banks + av 2 + mm 2 = 8.
"""

import numpy as np
import ml_dtypes

import concourse.bacc as bacc
import concourse.mybir as mybir
import concourse.tile as tile
from concourse.bass_utils import run_bass_kernel_spmd
from concourse.masks import make_identity

BF16 = mybir.dt.bfloat16
F32 = mybir.dt.float32
AF = mybir.ActivationFunctionType
MULT = mybir.AluOpType.mult

B, S, D = 2, 2048, 2048
BS = B * S                    # 4096 tokens
NUM_HEADS, NUM_KV_HEADS, DH = 32, 8, 64
N_CORES = 8
HL = NUM_HEADS // N_CORES     # 4 local query heads
DQ = HL * DH                  # 256 local q dims
SC = S // 128                 # 16 key chunks per batch
SCALE = 1.0 / 8.0             # 1/sqrt(64)


def build_kernel(KT):
    """Build the per-core Bass program. KT = number of 128-row contraction
    tiles in the projections (16, or 17 when biases are folded in via an
    appended ones-row of x^T)."""
    nc = bacc.Bacc("TRN2", target_bir_lowering=False, debug=False,
                   num_devices=N_CORES)
    xt = nc.dram_tensor("xt", [KT * 128, BS], BF16, kind="ExternalInput").ap()
    wqkv = nc.dram_tensor("wqkv", [KT * 128, DQ + 2 * DH], BF16,
                          kind="ExternalInput").ap()
    wo = nc.dram_tensor("wo", [DQ, D], BF16, kind="ExternalInput").ap()
    yt = nc.dram_tensor("yt", [D, BS], BF16, kind="ExternalOutput").ap()

    xt_r = xt.rearrange("(kt p) n -> p kt n", p=128)
    wqkv_r = wqkv.rearrange("(kt p) m -> p kt m", p=128)
    wo_r = wo.rearrange("(kt p) n -> p kt n", p=128)

    with tile.TileContext(nc) as tc:
        with tc.tile_pool(name="singles", bufs=1) as singles, \
             tc.tile_pool(name="psum", bufs=1, space="PSUM") as psum, \
             tc.tile_pool(name="sbuf", bufs=1) as sbuf, \
             tc.tile_pool(name="xkpool", bufs=2) as xkpool, \
             tc.tile_pool(name="exppool", bufs=2) as exppool:

            # --- resident SBUF tensors -----------------------------------
            wqkv_sb = singles.tile([128, KT, DQ + 2 * DH], BF16, tag="wqkv")
            nc.sync.dma_start(wqkv_sb[:], wqkv_r[:])
            wo_sb = singles.tile([128, 2, D], BF16, tag="wo")
            nc.sync.dma_start(wo_sb[:], wo_r[:])

            # per-batch tensors (statically double-allocated)
            # q^T: m-tile 0 holds heads 0,1 (rows 0-63 / 64-127), m-tile 1
            # holds heads 2,3. k^T duplicated into both partition halves so
            # both row-tiles of the paired scores matmuls can read it.
            qt = [singles.tile([128, 2, S], BF16, tag=f"qt{b}",
                               name=f"qt{b}") for b in range(B)]
            kt2 = [singles.tile([128, S], BF16, tag=f"kt2{b}",
                                name=f"kt2{b}") for b in range(B)]
            # v in natural layout [token-chunk, 128]; columns 0-63 are all
            # 1.0 so the AV matmul lands 64 broadcast copies of the softmax
            # denominator on PSUM partitions 0-63 (base partition 0, which
            # the custom-DVE reciprocal requires); columns 64-127 hold the
            # 64 v dims, so the attention output lands on partitions 64-127.
            # Normalization then needs no PE broadcast, just a DVE
            # reciprocal + multiply.
            v_sb = [singles.tile([128, SC, 128], BF16, tag=f"v{b}",
                                 name=f"v{b}") for b in range(B)]
            ot = [singles.tile([128, 2, S], BF16, tag=f"ot{b}",
                               name=f"ot{b}") for b in range(B)]
            for b in range(B):
                for kt in range(SC):
                    nc.vector.memset(v_sb[b][:, kt, 0:DH], 1.0)

            ident = singles.tile([DH, DH], BF16, tag="ident")
            make_identity(nc, ident)

            # --- emission helpers ----------------------------------------
            def emit_proj_block(b, blk):
                """Projections for one 512-token block of batch b."""
                gcols = slice(b * S + blk * 512, b * S + (blk + 1) * 512)
                cols = slice(blk * 512, (blk + 1) * 512)
                xk = xkpool.tile([128, KT, 512], BF16, tag="xk")
                nc.sync.dma_start(xk[:], xt_r[:, :, gcols])
                for m in range(3):
                    pp = psum.tile([128, 512], F32, tag="mm", bufs=2)
                    msl = slice(m * 128, (m + 1) * 128)
                    for kt in range(KT):
                        nc.tensor.matmul(
                            pp[:], lhsT=wqkv_sb[:, kt, msl],
                            rhs=xk[:, kt, :],
                            start=(kt == 0), stop=(kt == KT - 1))
                    if m < 2:
                        nc.vector.tensor_copy(out=qt[b][:, m, cols],
                                              in_=pp[:])
                    else:
                        # rows 0-63: k^T (duplicate to both halves);
                        # rows 64-127: v^T (transpose into v_sb)
                        nc.vector.tensor_copy(out=kt2[b][0:64, cols],
                                              in_=pp[0:64, :])
                        nc.vector.tensor_copy(out=kt2[b][64:128, cols],
                                              in_=pp[0:64, :])
                        vt = sbuf.tile([64, 512], BF16, tag="vt", bufs=2)
                        nc.vector.tensor_copy(out=vt[:], in_=pp[64:128, :])
                        for j in range(4):
                            pt = psum.tile([128, DH], BF16, tag="mm",
                                           bufs=2)
                            nc.tensor.transpose(
                                pt[:], vt[:, j * 128:(j + 1) * 128],
                                ident[:])
                            nc.vector.tensor_copy(
                                out=v_sb[b][:, blk * 4 + j, DH:],
                                in_=pt[:])

            def emit_attn_unit(b, m, qb):
                """Attention for 512 queries of head-pair m, batch b."""
                qcols = slice(qb * 512, (qb + 1) * 512)
                exps = exppool.tile([128, SC, 2, 512], BF16, tag="exps")
                avs = [psum.tile([128, 512], F32, tag="av", bufs=2,
                                 name=f"av{h}") for h in range(2)]
                def emit_av(kt):
                    for h in range(2):
                        nc.tensor.matmul(
                            avs[h][:],
                            lhsT=v_sb[b][:, kt, :],
                            rhs=exps[:, kt, h, :],
                            start=(kt == 0), stop=(kt == SC - 1))

                for kt in range(SC):
                    kr = slice(kt * 128, (kt + 1) * 128)
                    # paired scores matmuls: h0 on rows 0-63 (tile T0),
                    # h1 on rows 64-127 (tile T8) — concurrent row-tiles.
                    sc = psum.tile([128, 2, 512], F32, tag="sc", bufs=2)
                    for h in range(2):
                        rows = slice(h * 64, (h + 1) * 64)
                        nc.tensor.matmul(
                            sc[:, h, :], lhsT=kt2[b][rows, kr],
                            rhs=qt[b][rows, m, qcols],
                            start=True, stop=True)
                    # one exp for both heads: [128, 1024] PSUM -> SBUF
                    nc.scalar.activation(
                        out=exps[:, kt, :, :], in_=sc[:, :, :],
                        func=AF.Exp, scale=SCALE)
                    # AV lags one chunk so the PE FIFO never blocks on the
                    # exp it just requested.
                    if kt > 0:
                        emit_av(kt - 1)
                emit_av(SC - 1)
                # normalization: 64 broadcast copies of the denominator sit
                # on partitions 0-63 of each AV accumulator, the attention
                # output on partitions 64-127.
                for h in range(2):
                    rows = slice(h * 64, (h + 1) * 64)
                    rec = sbuf.tile([64, 512], F32, tag="rec", bufs=2,
                                    name=f"rec{h}")
                    nc.vector.reciprocal_approx_fast(rec[:],
                                                     avs[h][0:64, :])
                    nc.vector.tensor_tensor(
                        out=ot[b][rows, m, qcols],
                        in0=avs[h][64:128, :], in1=rec[:],
                        op=MULT)

            def emit_oproj(b, qb):
                """Output projection for 512 queries of batch b (needs both
                head-pairs of this query block in ot)."""
                qcols = slice(qb * 512, (qb + 1) * 512)
                gcols = slice(b * S + qb * 512, b * S + (qb + 1) * 512)
                for dm in range(16):
                    dsl = slice(dm * 128, (dm + 1) * 128)
                    po = psum.tile([128, 512], F32, tag="mm", bufs=2)
                    for kt in range(2):
                        nc.tensor.matmul(
                            po[:], lhsT=wo_sb[:, kt, dsl],
                            rhs=ot[b][:, kt, qcols],
                            start=(kt == 0), stop=(kt == 1))
                    oc = sbuf.tile([128, 512], BF16, tag="oc", bufs=4)
                    nc.vector.tensor_copy(out=oc[:], in_=po[:])
                    nc.sync.dma_start(yt[dsl, gcols], oc[:])

            # --- pipelined schedule --------------------------------------
            for blk in range(4):
                emit_proj_block(0, blk)
            for qb in range(4):
                emit_attn_unit(0, 0, qb)
                emit_proj_block(1, qb)
            for qb in range(4):
                emit_attn_unit(0, 1, qb)
                emit_oproj(0, qb)
            for qb in range(4):
                emit_attn_unit(1, 0, qb)
            for qb in range(4):
                emit_attn_unit(1, 1, qb)
                emit_oproj(1, qb)
    nc.compile()
    return nc


_CACHE = {}


def _get_kernel(KT):
    if KT not in _CACHE:
        _CACHE[KT] = build_kernel(KT)
    return _CACHE[KT]


def kernel(x, Wq, bq, Wk, bk, Wv, bv, Wo, bo):
    x = np.asarray(x, dtype=np.float32)
    Wq = np.asarray(Wq, dtype=np.float32)
    Wk = np.asarray(Wk, dtype=np.float32)
    Wv = np.asarray(Wv, dtype=np.float32)
    Wo = np.asarray(Wo, dtype=np.float32)
    bq = np.asarray(bq, dtype=np.float32)
    bk = np.asarray(bk, dtype=np.float32)
    bv = np.asarray(bv, dtype=np.float32)
    bo = np.asarray(bo, dtype=np.float32)
    assert x.shape == (B, S, D)

    has_bias = bool(np.any(bq) or np.any(bk) or np.any(bv))
    KT = 17 if has_bias else 16

    # x^T [D, B*S] (+ ones row when biases are folded into the projections)
    xt = np.zeros((KT * 128, BS), dtype=ml_dtypes.bfloat16)
    xt[:D] = x.transpose(2, 0, 1).reshape(D, BS).astype(ml_dtypes.bfloat16)
    if has_bias:
        xt[D] = 1.0

    nc = _get_kernel(KT)
    in_maps = []
    for c in range(N_CORES):
        qsl = slice(c * DQ, (c + 1) * DQ)
        ksl = slice(c * DH, (c + 1) * DH)
        wqkv = np.zeros((KT * 128, DQ + 2 * DH), dtype=ml_dtypes.bfloat16)
        wqkv[:D, :DQ] = Wq[:, qsl].astype(ml_dtypes.bfloat16)
        wqkv[:D, DQ:DQ + DH] = Wk[:, ksl].astype(ml_dtypes.bfloat16)
        wqkv[:D, DQ + DH:] = Wv[:, ksl].astype(ml_dtypes.bfloat16)
        if has_bias:
            wqkv[D, :DQ] = bq[qsl].astype(ml_dtypes.bfloat16)
            wqkv[D, DQ:DQ + DH] = bk[ksl].astype(ml_dtypes.bfloat16)
            wqkv[D, DQ + DH:] = bv[ksl].astype(ml_dtypes.bfloat16)
        in_maps.append({
            "xt": xt,
            "wqkv": wqkv,
            "wo": np.ascontiguousarray(Wo[qsl]).astype(ml_dtypes.bfloat16),
        })

    res = run_bass_kernel_spmd(nc, in_maps, core_ids=list(range(N_CORES)))
    yt = np.zeros((D, BS), dtype=np.float32)
    for r in res.results:
        yt += np.asarray(r["yt"], dtype=np.float32)
    y = yt.reshape(D, B, S).transpose(1, 2, 0) + bo
    return np.ascontiguousarray(y, dtype=np.float32)


# revision 18
# speedup vs baseline: 1.4309x; 1.0619x over previous
"""GQA attention kernel for Trainium2, sharded over 8 NeuronCores.

Problem: B=2, S=2048, D=2048, 32 query heads / 8 KV heads, head_dim=64.
Sharding: tensor-parallel over heads — core c owns query heads [4c, 4c+4)
and KV head c (exactly one GQA group), plus the matching slices of the
projections. Each core computes a partial of the output projection
(contraction over its 256 of the 2048 Wo rows); partials are summed on host.

On-device layout is fully "transposed": x arrives as x^T [D, B*S], the
projections produce q^T/k^T (token dim on the free axis), scores are computed
as S^T = K·q^T (so the softmax denominator is a partition-dim sum, obtained
for free from a ones-column appended to V), and the output projection emits
y^T = Wo^T · out^T. Softmax skips the max-subtraction (scores/8 ~ N(0,1); exp
cannot overflow). All matmuls run in bf16 with fp32 PSUM accumulation.

Schedule (v2): software-pipelined across batches — projections for batch 1
are emitted inside batch 0's attention, and each batch's output projection is
emitted inside its own m=1 attention pass. The two scores matmuls of a head
pair are adjacent 64x128 row-tiles (T0/T8) so they run concurrently on the PE
array; exp is one [128,1024] ACT instruction per key chunk covering both
heads. PSUM budget: sc 2x2 banks + av 2 + mm 2 = 8.
"""

import numpy as np
import ml_dtypes

import concourse.bacc as bacc
import concourse.mybir as mybir
import concourse.tile as tile
from concourse.bass_utils import run_bass_kernel_spmd
from concourse.masks import make_identity

BF16 = mybir.dt.bfloat16
F32 = mybir.dt.float32
AF = mybir.ActivationFunctionType
MULT = mybir.AluOpType.mult

B, S, D = 2, 2048, 2048
BS = B * S                    # 4096 tokens
NUM_HEADS, NUM_KV_HEADS, DH = 32, 8, 64
N_CORES = 8
HL = NUM_HEADS // N_CORES     # 4 local query heads
DQ = HL * DH                  # 256 local q dims
SC = S // 128                 # 16 key chunks per batch
SCALE = 1.0 / 8.0             # 1/sqrt(64)


def build_kernel(KT):
    """Build the per-core Bass program. KT = number of 128-row contraction
    tiles in the projections (16, or 17 when biases are folded in via an
    appended ones-row of x^T)."""
    nc = bacc.Bacc("TRN2", target_bir_lowering=False, debug=False,
                   num_devices=N_CORES)
    xt = nc.dram_tensor("xt", [KT * 128, BS], BF16, kind="ExternalInput").ap()
    wqkv = nc.dram_tensor("wqkv", [KT * 128, DQ + 2 * DH], BF16,
                          kind="ExternalInput").ap()
    wo = nc.dram_tensor("wo", [DQ, D], BF16, kind="ExternalInput").ap()
    yt = nc.dram_tensor("yt", [D, BS], BF16, kind="ExternalOutput").ap()

    xt_r = xt.rearrange("(kt p) n -> p kt n", p=128)
    wqkv_r = wqkv.rearrange("(kt p) m -> p kt m", p=128)
    wo_r = wo.rearrange("(kt p) n -> p kt n", p=128)

    with tile.TileContext(nc) as tc:
        with tc.tile_pool(name="singles", bufs=1) as singles, \
             tc.tile_pool(name="psum", bufs=1, space="PSUM") as psum, \
             tc.tile_pool(name="sbuf", bufs=1) as sbuf, \
             tc.tile_pool(name="xkpool", bufs=2) as xkpool, \
             tc.tile_pool(name="exppool", bufs=2) as exppool:

            # --- resident SBUF tensors -----------------------------------
            wqkv_sb = singles.tile([128, KT, DQ + 2 * DH], BF16, tag="wqkv")
            for m in range(3):
                msl = slice(m * 128, (m + 1) * 128)
                nc.sync.dma_start(wqkv_sb[:, :, msl], wqkv_r[:, :, msl])
            wo_sb = singles.tile([128, 2, D], BF16, tag="wo")
            nc.sync.dma_start(wo_sb[:], wo_r[:])

            # per-batch tensors (statically double-allocated)
            # q^T: m-tile 0 holds heads 0,1 (rows 0-63 / 64-127), m-tile 1
            # holds heads 2,3. k^T duplicated into both partition halves so
            # both row-tiles of the paired scores matmuls can read it.
            qt = [singles.tile([128, 2, S], BF16, tag=f"qt{b}",
                               name=f"qt{b}") for b in range(B)]
            kt2 = [singles.tile([128, S], BF16, tag=f"kt2{b}",
                                name=f"kt2{b}") for b in range(B)]
            # v in natural layout [token-chunk, 128]; columns 0-63 are all
            # 1.0 so the AV matmul lands 64 broadcast copies of the softmax
            # denominator on PSUM partitions 0-63 (base partition 0, which
            # the custom-DVE reciprocal requires); columns 64-127 hold the
            # 64 v dims, so the attention output lands on partitions 64-127.
            # Normalization then needs no PE broadcast, just a DVE
            # reciprocal + multiply.
            v_sb = [singles.tile([128, SC, 128], BF16, tag=f"v{b}",
                                 name=f"v{b}") for b in range(B)]
            ot = [singles.tile([128, 2, S], BF16, tag=f"ot{b}",
                               name=f"ot{b}") for b in range(B)]
            for b in range(B):
                for kt in range(SC):
                    nc.vector.memset(v_sb[b][:, kt, 0:DH], 1.0)

            ident = singles.tile([DH, DH], BF16, tag="ident")
            make_identity(nc, ident)

            # --- emission helpers ----------------------------------------
            xk_tiles = {}

            def emit_proj_part(b, blk, m):
                """One m-tile of the projections for one 512-token block.
                Parts are emitted in block-major order; the xk DMA is issued
                lazily (split in two so the first matmuls start sooner)."""
                gcols = slice(b * S + blk * 512, b * S + (blk + 1) * 512)
                cols = slice(blk * 512, (blk + 1) * 512)
                if (b, blk) not in xk_tiles:
                    xk = xkpool.tile([128, KT, 512], BF16, tag="xk")
                    half = KT // 2
                    nc.sync.dma_start(xk[:, 0:half, :],
                                      xt_r[:, 0:half, gcols])
                    nc.sync.dma_start(xk[:, half:, :],
                                      xt_r[:, half:, gcols])
                    xk_tiles[(b, blk)] = xk
                xk = xk_tiles[(b, blk)]
                pp = psum.tile([128, 512], F32, tag="mm", bufs=2)
                msl = slice(m * 128, (m + 1) * 128)
                for kt in range(KT):
                    nc.tensor.matmul(
                        pp[:], lhsT=wqkv_sb[:, kt, msl],
                        rhs=xk[:, kt, :],
                        start=(kt == 0), stop=(kt == KT - 1))
                if m < 2:
                    nc.vector.tensor_copy(out=qt[b][:, m, cols], in_=pp[:])
                else:
                    # rows 0-63: k^T (duplicate to both halves);
                    # rows 64-127: v^T (transpose into v_sb)
                    nc.vector.tensor_copy(out=kt2[b][0:64, cols],
                                          in_=pp[0:64, :])
                    nc.vector.tensor_copy(out=kt2[b][64:128, cols],
                                          in_=pp[0:64, :])
                    vt = sbuf.tile([64, 512], BF16, tag="vt", bufs=2)
                    nc.vector.tensor_copy(out=vt[:], in_=pp[64:128, :])
                    for j in range(4):
                        pt = psum.tile([128, DH], BF16, tag="mm",
                                       bufs=2)
                        nc.tensor.transpose(
                            pt[:], vt[:, j * 128:(j + 1) * 128],
                            ident[:])
                        nc.vector.tensor_copy(
                            out=v_sb[b][:, blk * 4 + j, DH:],
                            in_=pt[:])

            def emit_attn_unit(b, m, qb):
                """Attention for 512 queries of head-pair m, batch b."""
                qcols = slice(qb * 512, (qb + 1) * 512)
                exps = exppool.tile([128, SC, 2, 512], BF16, tag="exps")
                avs = [psum.tile([128, 512], F32, tag="av", bufs=2,
                                 name=f"av{h}") for h in range(2)]
                def emit_av(kt):
                    for h in range(2):
                        nc.tensor.matmul(
                            avs[h][:],
                            lhsT=v_sb[b][:, kt, :],
                            rhs=exps[:, kt, h, :],
                            start=(kt == 0), stop=(kt == SC - 1))

                # Process key chunks in pairs: both chunks' scores (64x128
                # row-tile mode), then the lagged AV matmuls (128x128 mode)
                # — halves the PE tiling-mode switches. AV lags one pair so
                # the PE FIFO never blocks on the exps it just requested.
                for kt0 in range(0, SC, 2):
                    for kt in (kt0, kt0 + 1):
                        kr = slice(kt * 128, (kt + 1) * 128)
                        # paired scores matmuls: h0 on rows 0-63 (tile T0),
                        # h1 on rows 64-127 (tile T8) — concurrent.
                        sc = psum.tile([128, 2, 512], F32, tag="sc", bufs=2)
                        for h in range(2):
                            rows = slice(h * 64, (h + 1) * 64)
                            nc.tensor.matmul(
                                sc[:, h, :], lhsT=kt2[b][rows, kr],
                                rhs=qt[b][rows, m, qcols],
                                start=True, stop=True)
                        # one exp for both heads: [128,1024] PSUM -> SBUF
                        nc.scalar.activation(
                            out=exps[:, kt, :, :], in_=sc[:, :, :],
                            func=AF.Exp, scale=SCALE)
                    if kt0 > 0:
                        emit_av(kt0 - 2)
                        emit_av(kt0 - 1)
                emit_av(SC - 2)
                emit_av(SC - 1)
                # normalization: 64 broadcast copies of the denominator sit
                # on partitions 0-63 of each AV accumulator, the attention
                # output on partitions 64-127.
                for h in range(2):
                    rows = slice(h * 64, (h + 1) * 64)
                    rec = sbuf.tile([64, 512], F32, tag="rec", bufs=2,
                                    name=f"rec{h}")
                    nc.vector.reciprocal_approx_fast(rec[:],
                                                     avs[h][0:64, :])
                    nc.vector.tensor_tensor(
                        out=ot[b][rows, m, qcols],
                        in0=avs[h][64:128, :], in1=rec[:],
                        op=MULT)

            def emit_oproj(b, qb):
                """Output projection for 512 queries of batch b (needs both
                head-pairs of this query block in ot)."""
                qcols = slice(qb * 512, (qb + 1) * 512)
                gcols = slice(b * S + qb * 512, b * S + (qb + 1) * 512)
                for dm in range(16):
                    dsl = slice(dm * 128, (dm + 1) * 128)
                    po = psum.tile([128, 512], F32, tag="mm", bufs=2)
                    for kt in range(2):
                        nc.tensor.matmul(
                            po[:], lhsT=wo_sb[:, kt, dsl],
                            rhs=ot[b][:, kt, qcols],
                            start=(kt == 0), stop=(kt == 1))
                    oc = sbuf.tile([128, 512], BF16, tag="oc", bufs=4)
                    nc.vector.tensor_copy(out=oc[:], in_=po[:])
                    nc.sync.dma_start(yt[dsl, gcols], oc[:])

            # --- pipelined schedule --------------------------------------
            # Batch-0 projections up front; batch-1 projection parts spread
            # evenly across all 8 batch-0 attention units; each batch's
            # output projection spread across the following 4 units so the
            # PE load per ACT-bound attention unit stays even.
            for blk in range(4):
                for m in range(3):
                    emit_proj_part(0, blk, m)
            b1_parts = [(blk, m) for blk in range(4) for m in range(3)]
            sched = [2, 1, 2, 1, 2, 1, 2, 1]  # parts per batch-0 unit
            for u, (mi, qb) in enumerate([(mi, qb) for mi in range(2)
                                          for qb in range(4)]):
                emit_attn_unit(0, mi, qb)
                for _ in range(sched[u]):
                    blk, m = b1_parts.pop(0)
                    emit_proj_part(1, blk, m)
            for qb in range(4):
                emit_attn_unit(1, 0, qb)
                emit_oproj(0, qb)
            for qb in range(4):
                emit_attn_unit(1, 1, qb)
                emit_oproj(1, qb)
    nc.compile()
    return nc


_CACHE = {}


def _get_kernel(KT):
    if KT not in _CACHE:
        _CACHE[KT] = build_kernel(KT)
    return _CACHE[KT]


def kernel(x, Wq, bq, Wk, bk, Wv, bv, Wo, bo):
    x = np.asarray(x, dtype=np.float32)
    Wq = np.asarray(Wq, dtype=np.float32)
    Wk = np.asarray(Wk, dtype=np.float32)
    Wv = np.asarray(Wv, dtype=np.float32)
    Wo = np.asarray(Wo, dtype=np.float32)
    bq = np.asarray(bq, dtype=np.float32)
    bk = np.asarray(bk, dtype=np.float32)
    bv = np.asarray(bv, dtype=np.float32)
    bo = np.asarray(bo, dtype=np.float32)
    assert x.shape == (B, S, D)

    has_bias = bool(np.any(bq) or np.any(bk) or np.any(bv))
    KT = 17 if has_bias else 16

    # x^T [D, B*S] (+ ones row when biases are folded into the projections)
    xt = np.zeros((KT * 128, BS), dtype=ml_dtypes.bfloat16)
    xt[:D] = x.transpose(2, 0, 1).reshape(D, BS).astype(ml_dtypes.bfloat16)
    if has_bias:
        xt[D] = 1.0

    nc = _get_kernel(KT)
    in_maps = []
    for c in range(N_CORES):
        qsl = slice(c * DQ, (c + 1) * DQ)
        ksl = slice(c * DH, (c + 1) * DH)
        wqkv = np.zeros((KT * 128, DQ + 2 * DH), dtype=ml_dtypes.bfloat16)
        wqkv[:D, :DQ] = Wq[:, qsl].astype(ml_dtypes.bfloat16)
        wqkv[:D, DQ:DQ + DH] = Wk[:, ksl].astype(ml_dtypes.bfloat16)
        wqkv[:D, DQ + DH:] = Wv[:, ksl].astype(ml_dtypes.bfloat16)
        if has_bias:
            wqkv[D, :DQ] = bq[qsl].astype(ml_dtypes.bfloat16)
            wqkv[D, DQ:DQ + DH] = bk[ksl].astype(ml_dtypes.bfloat16)
            wqkv[D, DQ + DH:] = bv[ksl].astype(ml_dtypes.bfloat16)
        in_maps.append({
            "xt": xt,
            "wqkv": wqkv,
            "wo": np.ascontiguousarray(Wo[qsl]).astype(ml_dtypes.bfloat16),
        })

    res = run_bass_kernel_spmd(nc, in_maps, core_ids=list(range(N_CORES)))
    yt = np.zeros((D, BS), dtype=np.float32)
    for r in res.results:
        yt += np.asarray(r["yt"], dtype=np.float32)
    y = yt.reshape(D, B, S).transpose(1, 2, 0) + bo
    return np.ascontiguousarray(y, dtype=np.float32)
